# revision 2
# baseline (speedup 1.0000x reference)
"""ChannelWiseFloat8GroupedLinear — expert-parallel Trainium2 Bass kernel.

Problem: x [8192, 1024] f32, weight [8*1024, 1024] f32, tokens_per_expert
[8] int32 (uniform 1024).  out[t, d] = x_dq @ w_dq[e(t)].T in bf16, where
x is fp8-e4m3fn quant-dequantized per token row and w per expert block.

Sharding: expert-parallel over 8 NeuronCores.  Tokens are contiguous per
expert (cumsum offsets), so core e owns x rows [1024e, 1024e+1024) and
expert e's weight block — no cross-core communication.

Device math: the reference quantizes to OCP e4m3fn (max 448); TRN2's
fp8_e4m3 tops out at 240.  Quantizing with r = 224/amax instead of
448/amax lands on the halved e4m3fn grid, which TRN e4m3 represents
exactly (up to a negligible subnormal-spacing difference), and the x4
is folded into the output scale m[t] = amax_x[t]*amax_w/(448*448/4).
The fp8 matmul accumulates exact products in f32 PSUM, so the result
matches the reference to ~f32 rounding before the final bf16 cast.

Schedule (v2): all input DMAs ride the sync HWDGE ring w-first (ring is
FIFO, so w gets full bandwidth, then x streams behind it).  While w
loads, PE transposes it in f32 and ACT evicts to SBUF; DVE reduces the
global |w| amax.  Once rw is known, w quant is split DVE/ACT/GPSIMD per
k-slice and the main matmul consumes k-pairs in quant-completion order.
x tiles pipeline per-tile: DVE amax -> GPSIMD quant -> PE fp8 transpose
-> DVE/ACT evict -> DoubleRow fp8 matmul -> scaled bf16 evict -> store.
"""

import numpy as np
import ml_dtypes

P = 128
TPE = 1024   # tokens per expert (= T // ne, uniform)
DIN = 1024
DOUT = 1024
NE = 8
NT = TPE // P    # 8 token tiles per core
ND = DOUT // P   # 8 dout tiles per core
NK = DIN // P    # 8 contraction tiles
E4M3_MAX = 448.0
EPS = 1e-12

_CACHE = {}


def _axon_device_reset():
    """Best-effort reset of the axon-tunneled NeuronCores after an
    NRT_EXEC_UNIT_UNRECOVERABLE wedge (observed rarely; a reset recovers)."""
    try:
        import ctypes

        import jax

        jax.devices()
        lib = ctypes.CDLL("/opt/axon/libaxon_pjrt.so")
        if hasattr(lib, "axon_reset"):
            lib.axon_reset.restype = ctypes.c_int64
            lib.axon_reset()
    except Exception:
        pass


def _build_nc():
    """Build + compile the single-core Bass program (run SPMD on 8 cores)."""
    import concourse.mybir as mybir
    import concourse.tile as tile
    from concourse import bacc, bass_isa
    from concourse.masks import make_identity

    dt = mybir.dt
    X = mybir.AxisListType.X
    ALU = mybir.AluOpType
    DR = mybir.MatmulPerfMode.DoubleRow

    nc = bacc.Bacc("TRN2", target_bir_lowering=False, debug=False)
    x_t = nc.dram_tensor("x", [TPE, DIN], dt.float32, kind="ExternalInput")
    w_t = nc.dram_tensor("w", [DOUT, DIN], dt.float32, kind="ExternalInput")
    o_t = nc.dram_tensor("o", [TPE, DOUT], dt.bfloat16, kind="ExternalOutput")

    x_d = x_t.ap().rearrange("(tt p) k -> p tt k", p=P)   # [128, 8, 1024]
    w_d = w_t.ap().rearrange("(dd p) k -> p dd k", p=P)
    o_d = o_t.ap().rearrange("(tt p) d -> p tt d", p=P)

    with tile.TileContext(nc) as tc:
        with (
            tc.tile_pool(name="const", bufs=1) as const,
            tc.tile_pool(name="big", bufs=1) as big,
            tc.tile_pool(name="small", bufs=1) as small,
            tc.tile_pool(name="outp", bufs=3) as outp,
            tc.tile_pool(name="pt", bufs=2, space="PSUM") as pt,
            tc.tile_pool(name="pm", bufs=2, space="PSUM") as pm,
        ):
            # persistent buffers
            x_sb = big.tile([P, NT, DIN], dt.float32, tag="x_sb")
            w_sb = big.tile([P, ND, DIN], dt.float32, tag="w_sb")
            wT = big.tile([P, NK, ND, P], dt.float32, tag="wT")
            qx = big.tile([P, NT, DIN], dt.float8e4, tag="qx")
            qxT = big.tile([P, NK, NT, P], dt.float8e4, tag="qxT")
            qwT = big.tile([P, NK, ND, P], dt.float8e4, tag="qwT")

            amw_parts = small.tile([P, ND], dt.float32, tag="amw_parts")
            amw_c = small.tile([P, 1], dt.float32, tag="amw_c")
            amw_g = small.tile([P, 1], dt.float32, tag="amw_g")
            inv_w = small.tile([P, 1], dt.float32, tag="inv_w")
            rw = small.tile([P, 1], dt.float32, tag="rw")
            cw = small.tile([P, 1], dt.float32, tag="cw")
            amx_parts = small.tile([P, NT], dt.float32, tag="amx_parts")
            amx_cl = small.tile([P, NT], dt.float32, tag="amx_cl")
            inv_x = small.tile([P, NT], dt.float32, tag="inv_x")
            rx = small.tile([P, NT], dt.float32, tag="rx")
            m_all = small.tile([P, NT], dt.float32, tag="m_all")

            # --- input DMAs, all on the sync HWDGE ring: w first, then x.
            # The ring drains in FIFO order, so w gets full bandwidth and x
            # streams in right behind it without stealing from w. ---
            for i in range(ND):
                nc.sync.dma_start(w_sb[:, i, :], w_d[:, i, :])
            for i in range(NT):
                nc.sync.dma_start(x_sb[:, i, :], x_d[:, i, :])

            # transpose identities (fp8: 1.0 is exactly representable)
            id_f32 = const.tile([P, P], dt.float32, tag="id32f")
            make_identity(nc, id_f32[:])
            id_fp8 = const.tile([P, P], dt.float8e4, tag="id8")
            nc.vector.tensor_copy(id_fp8[:], id_f32[:])

            # --- w amax on DVE as tiles land: pairs, then singles for the
            # last two tiles so rw resolves as early as possible ---
            for p0 in range(3):
                nc.vector.reduce_max(
                    amw_parts[:, 2 * p0 : 2 * p0 + 2],
                    w_sb[:, 2 * p0 : 2 * p0 + 2, :],
                    axis=X,
                    apply_absolute_value=True,
                )
            for dd in (6, 7):
                nc.vector.reduce_max(
                    amw_parts[:, dd : dd + 1],
                    w_sb[:, dd, :],
                    axis=X,
                    apply_absolute_value=True,
                )

            # --- w: exact f32 transpose (PE transpose-mode) during the load
            # window (no amax dependency), staged to wT in f32 via ACT ---
            for dd in range(ND):
                pwf = pt.tile([P, NK, P], dt.float32, tag="pt")
                for kk in range(NK):
                    nc.tensor.transpose(
                        pwf[:, kk, :], w_sb[:, dd, kk * P : (kk + 1) * P], id_f32[:]
                    )
                nc.scalar.copy(wT[:, :, dd, :], pwf[:])

            # --- rw chain ---
            nc.vector.reduce_max(amw_c[:], amw_parts[:], axis=X)
            nc.vector.tensor_scalar_max(amw_c[:], amw_c[:], EPS)
            nc.gpsimd.partition_all_reduce(
                amw_g[:], amw_c[:], channels=P, reduce_op=bass_isa.ReduceOp.max
            )
            nc.vector.reciprocal(inv_w[:], amw_g[:])
            nc.vector.tensor_scalar_mul(rw[:], inv_w[:], E4M3_MAX / 2.0)
            nc.vector.tensor_scalar_mul(cw[:], amw_g[:], 4.0 / (E4M3_MAX * E4M3_MAX))

            # --- w quant split across engines; main matmul consumes k-pairs
            # in completion order: (0,1) DVE, (4,5) GPSIMD, (2,3) ACT,
            # (6,7) GPSIMD-late ---
            for kk in (0, 1):
                nc.vector.tensor_scalar_mul(qwT[:, kk, :, :], wT[:, kk, :, :], rw[:])
            for kk in (2, 3):
                nc.scalar.mul(qwT[:, kk, :, :], wT[:, kk, :, :], rw[:])
            for kk in (4, 5):
                nc.gpsimd.tensor_scalar_mul(qwT[:, kk, :, :], wT[:, kk, :, :], rw[:])

            # --- x chain.  amax + minis on DVE (singles for t0/t1 so the
            # pipe starts early, pairs after); quant on GPSIMD; fp8 PE
            # transpose; evict alternating DVE/ACT. ---
            def x_minis(sl):
                nc.vector.tensor_scalar_max(amx_cl[:, sl], amx_parts[:, sl], EPS)
                nc.vector.reciprocal(inv_x[:, sl], amx_cl[:, sl])
                nc.vector.tensor_scalar_mul(rx[:, sl], inv_x[:, sl], E4M3_MAX / 2.0)
                nc.vector.tensor_scalar(
                    m_all[:, sl], amx_cl[:, sl], cw[:], None, op0=ALU.mult
                )

            def x_quant(tt):
                nc.gpsimd.tensor_scalar_mul(
                    qx[:, tt, :], x_sb[:, tt, :], rx[:, tt : tt + 1]
                )

            def x_transpose(tt):
                pxf = pt.tile([P, NK, P], dt.float32, tag="pt")
                for kk in range(NK):
                    nc.tensor.matmul(
                        pxf[:, kk, :],
                        lhsT=qx[:, tt, kk * P : (kk + 1) * P],
                        rhs=id_fp8[:],
                        start=True, stop=True,
                    )
                return pxf

            def x_evict(tt, pxf):
                if tt % 2 == 0:
                    nc.scalar.copy(qxT[:, :, tt, :], pxf[:])
                else:
                    nc.vector.tensor_copy(qxT[:, :, tt, :], pxf[:])

            # t0, t1 singles
            for tt in (0, 1):
                nc.vector.reduce_max(
                    amx_parts[:, tt : tt + 1],
                    x_sb[:, tt, :],
                    axis=X,
                    apply_absolute_value=True,
                )
                x_minis(slice(tt, tt + 1))
            # late w quant on gpsimd, between x quants t1 and t2
            x_quant(0)
            x_quant(1)
            for kk in (6, 7):
                nc.gpsimd.tensor_scalar_mul(qwT[:, kk, :, :], wT[:, kk, :, :], rw[:])
            pxf0 = x_transpose(0)
            x_evict(0, pxf0)
            pxf1 = x_transpose(1)
            x_evict(1, pxf1)
            # pairs for t2..t7
            for pr in range(1, 4):
                t0 = 2 * pr
                sl = slice(t0, t0 + 2)
                nc.vector.reduce_max(
                    amx_parts[:, sl],
                    x_sb[:, sl, :],
                    axis=X,
                    apply_absolute_value=True,
                )
                x_minis(sl)
                for tt in (t0, t0 + 1):
                    x_quant(tt)
                    pxf = x_transpose(tt)
                    x_evict(tt, pxf)

            # --- main fp8 DoubleRow matmul: out[t,d] accumulated over k in
            # quant-completion order of the k-pairs ---
            KP_ORDER = (0, 2, 1, 3)   # k-pairs (0,1), (4,5), (2,3), (6,7)
            for tt in range(NT):
                po = pm.tile([P, DOUT], dt.float32, tag="pm")
                for i, kp in enumerate(KP_ORDER):
                    ks = slice(2 * kp, 2 * kp + 2)
                    st, sp = i == 0, i == len(KP_ORDER) - 1
                    for h in range(2):
                        nc.tensor.matmul(
                            po[:, h * 512 : (h + 1) * 512],
                            lhsT=qxT[:, ks, tt, :],
                            rhs=qwT[:, ks, 4 * h : 4 * h + 4, :],
                            start=st, stop=sp,
                            perf_mode=DR,
                        )
                ob = outp.tile([P, DOUT], dt.bfloat16, tag="ob")
                if tt % 2 == 0:
                    nc.vector.tensor_scalar_mul(ob[:], po[:], m_all[:, tt : tt + 1])
                else:
                    nc.scalar.mul(ob[:], po[:], m_all[:, tt : tt + 1])
                nc.sync.dma_start(o_d[:, tt, :], ob[:])

    nc.compile()
    return nc


def get_nc():
    if "nc" not in _CACHE:
        _CACHE["nc"] = _build_nc()
    return _CACHE["nc"]


def make_in_maps(x, weight):
    x = np.ascontiguousarray(np.asarray(x, dtype=np.float32))
    w = np.ascontiguousarray(np.asarray(weight, dtype=np.float32))
    return [
        {"x": x[TPE * e : TPE * (e + 1)], "w": w[DOUT * e : DOUT * (e + 1)]}
        for e in range(NE)
    ]


def _host_reference(x, weight, tokens_per_expert):
    """Exact numpy port of the reference — fallback for non-uniform routing."""
    x = np.asarray(x, dtype=np.float32)
    w = np.asarray(weight, dtype=np.float32)
    tpe = np.asarray(tokens_per_expert, dtype=np.int64)
    ne = tpe.shape[0]
    T, din = x.shape
    dout = w.shape[0] // ne
    wr = w.reshape(ne, dout, din)

    def qd(v, axis, fmax):
        amax = np.max(np.abs(v), axis=axis, keepdims=True)
        scale = np.maximum(amax, EPS) / fmax
        q = np.clip(v / scale, -fmax, fmax).astype(ml_dtypes.float8_e4m3fn)
        return q.astype(np.float32) * scale

    w_dq = qd(wr, (1, 2), E4M3_MAX)
    x_dq = qd(x, -1, E4M3_MAX)
    offs = np.cumsum(tpe)
    starts = offs - tpe
    out = np.zeros((T, dout), np.float32)
    for e in range(ne):
        s, t = int(starts[e]), int(offs[e])
        if t > s:
            out[s:t] = x_dq[s:t] @ w_dq[e].T
    return out.astype(ml_dtypes.bfloat16)


def kernel(x, weight, tokens_per_expert):
    x = np.asarray(x)
    weight = np.asarray(weight)
    tpe = np.asarray(tokens_per_expert)
    uniform = (
        x.shape == (NE * TPE, DIN)
        and weight.shape == (NE * DOUT, DIN)
        and tpe.shape == (NE,)
        and bool(np.all(tpe.astype(np.int64) == TPE))
    )
    if not uniform:
        return _host_reference(x, weight, tpe)

    from concourse.bass_utils import run_bass_kernel_spmd

    nc = get_nc()
    in_maps = make_in_maps(x, weight)
    try:
        res = run_bass_kernel_spmd(nc, in_maps, core_ids=list(range(NE)))
    except Exception:
        # rare device wedge (NRT_EXEC_UNIT_UNRECOVERABLE) — reset and retry
        _axon_device_reset()
        res = run_bass_kernel_spmd(nc, in_maps, core_ids=list(range(NE)))
    return np.concatenate([res.results[e]["o"] for e in range(NE)], axis=0)


if __name__ == "__main__":
    rng = np.random.default_rng(0)
    x = rng.standard_normal((NE * TPE, DIN), dtype=np.float32)
    w = (rng.standard_normal((NE * DOUT, DIN), dtype=np.float32) * 0.02).astype(
        np.float32
    )
    tpe = np.full((NE,), TPE, dtype=np.int32)
    out = kernel(x, w, tpe)
    exp = _host_reference(x, w, tpe)
    a = out.astype(np.float64)
    b = exp.astype(np.float64)
    denom = max(np.abs(b).max(), 1e-30)
    print("absmax rel err:", np.abs(a - b).max() / denom)
    rms = np.sqrt(((a - b) ** 2).mean()) / np.sqrt((b**2).mean())
    print("rms rel err:", rms)


# revision 10
# speedup vs baseline: 2.8081x; 2.8081x over previous
"""ChannelWiseFloat8GroupedLinear — expert-parallel Trainium2 Bass kernel.

Problem: x [8192, 1024] f32, weight [8*1024, 1024] f32, tokens_per_expert
[8] int32 (uniform 1024).  out[t, d] = x_dq @ w_dq[e(t)].T in bf16, where
x is fp8-e4m3fn quant-dequantized per token row and w per expert block.

Sharding: expert-parallel over 8 NeuronCores.  Tokens are contiguous per
expert (cumsum offsets), so core e owns x rows [1024e, 1024e+1024) and
expert e's weight block — no cross-core communication.

Device math: the reference quantizes to OCP e4m3fn (max 448); TRN2's
fp8_e4m3 tops out at 240.  Quantizing with r = 224/amax instead of
448/amax lands on the halved e4m3fn grid, which TRN e4m3 represents
exactly (up to a negligible subnormal-spacing difference), and the x4
is folded into the output scale m[t] = amax_x[t]*amax_w/(448*448/4).
The fp8 matmul accumulates exact products in f32 PSUM, so the result
matches the reference to ~f32 rounding before the final bf16 cast.

Schedule (v2): all input DMAs ride the sync HWDGE ring w-first (ring is
FIFO, so w gets full bandwidth, then x streams behind it).  While w
loads, PE transposes it in f32 and ACT evicts to SBUF; DVE reduces the
global |w| amax.  Once rw is known, w quant is split DVE/ACT/GPSIMD per
k-slice and the main matmul consumes k-pairs in quant-completion order.
x tiles pipeline per-tile: DVE amax -> GPSIMD quant -> PE fp8 transpose
-> DVE/ACT evict -> DoubleRow fp8 matmul -> scaled bf16 evict -> store.
"""

import numpy as np
import ml_dtypes

P = 128
TPE = 1024   # tokens per expert (= T // ne, uniform)
DIN = 1024
DOUT = 1024
NE = 8
NT = TPE // P    # 8 token tiles per core
ND = DOUT // P   # 8 dout tiles per core
NK = DIN // P    # 8 contraction tiles
E4M3_MAX = 448.0
EPS = 1e-12

_CACHE = {}


def _axon_device_reset():
    """Best-effort reset of the axon-tunneled NeuronCores after an
    NRT_EXEC_UNIT_UNRECOVERABLE wedge (observed rarely; a reset recovers)."""
    try:
        import ctypes

        import jax

        jax.devices()
        lib = ctypes.CDLL("/opt/axon/libaxon_pjrt.so")
        if hasattr(lib, "axon_reset"):
            lib.axon_reset.restype = ctypes.c_int64
            lib.axon_reset()
    except Exception:
        pass


def _build_nc():
    """Build + compile the single-core Bass program (run SPMD on 8 cores)."""
    import concourse.mybir as mybir
    import concourse.tile as tile
    from concourse import bacc, bass_isa
    from concourse.masks import make_identity

    dt = mybir.dt
    X = mybir.AxisListType.X
    ALU = mybir.AluOpType
    DR = mybir.MatmulPerfMode.DoubleRow

    nc = bacc.Bacc("TRN2", target_bir_lowering=False, debug=False)
    x_t = nc.dram_tensor("x", [TPE, DIN], dt.float32, kind="ExternalInput")
    w_t = nc.dram_tensor("w", [DOUT, DIN], dt.float32, kind="ExternalInput")
    o_t = nc.dram_tensor("o", [TPE, DOUT], dt.bfloat16, kind="ExternalOutput")

    x_d = x_t.ap().rearrange("(tt p) k -> p tt k", p=P)   # [128, 8, 1024]
    w_d = w_t.ap().rearrange("(dd p) k -> p dd k", p=P)
    o_d = o_t.ap().rearrange("(tt p) d -> p tt d", p=P)

    with tile.TileContext(nc) as tc:
        with (
            tc.tile_pool(name="const", bufs=1) as const,
            tc.tile_pool(name="big", bufs=1) as big,
            tc.tile_pool(name="small", bufs=1) as small,
            tc.tile_pool(name="outp", bufs=3) as outp,
            tc.tile_pool(name="pt", bufs=2, space="PSUM") as pt,
            tc.tile_pool(name="pm", bufs=2, space="PSUM") as pm,
        ):
            # persistent buffers
            x_sb = big.tile([P, NT, DIN], dt.float32, tag="x_sb")
            w_sb = big.tile([P, ND, DIN], dt.float32, tag="w_sb")
            wT = big.tile([P, NK, ND, P], dt.float32, tag="wT")
            qx = big.tile([P, NT, DIN], dt.float8e4, tag="qx")
            qxT = big.tile([P, NK, NT, P], dt.float8e4, tag="qxT")
            qwT = big.tile([P, NK, ND, P], dt.float8e4, tag="qwT")

            amw_parts = small.tile([P, ND], dt.float32, tag="amw_parts")
            amw_c = small.tile([P, 1], dt.float32, tag="amw_c")
            amw_g = small.tile([P, 1], dt.float32, tag="amw_g")
            inv_w = small.tile([P, 1], dt.float32, tag="inv_w")
            rw = small.tile([P, 1], dt.float32, tag="rw")
            cw = small.tile([P, 1], dt.float32, tag="cw")
            amx_parts = small.tile([P, NT], dt.float32, tag="amx_parts")
            amx_cl = small.tile([P, NT], dt.float32, tag="amx_cl")
            inv_x = small.tile([P, NT], dt.float32, tag="inv_x")
            rx = small.tile([P, NT], dt.float32, tag="rx")
            m_all = small.tile([P, NT], dt.float32, tag="m_all")
            junk = small.tile([P, 1], dt.float32, tag="junk")

            # --- input DMAs.  A single HWDGE ring only sustains ~250 GB/s
            # (FIFO gap between consecutive DMAs), so split each stream
            # across both HWDGE rings (sync + scalar).  Each ring is FIFO,
            # so w transfers strictly precede x transfers per ring; x-odds
            # ride SWDGE (gpsimd), gated on w completion by a cheap native
            # partition_all_reduce that reads the last w tile. ---
            for i in (0, 2, 4, 6):
                nc.sync.dma_start(w_sb[:, i, :], w_d[:, i, :])
            for i in (1, 3, 5, 7):
                nc.scalar.dma_start(w_sb[:, i, :], w_d[:, i, :])
            for i in (0, 2, 4, 6):
                nc.sync.dma_start(x_sb[:, i, :], x_d[:, i, :])

            # transpose identities (fp8: 1.0 is exactly representable)
            id_f32 = const.tile([P, P], dt.float32, tag="id32f")
            make_identity(nc, id_f32[:])
            id_fp8 = const.tile([P, P], dt.float8e4, tag="id8")
            nc.vector.tensor_copy(id_fp8[:], id_f32[:])

            # x-odd loads on SWDGE, gated behind the last w tile
            nc.gpsimd.partition_all_reduce(
                junk[:], w_sb[:, 7, 0:1], channels=P,
                reduce_op=bass_isa.ReduceOp.max,
            )
            for i in (1, 3, 5, 7):
                nc.gpsimd.dma_start(x_sb[:, i, :], x_d[:, i, :])

            # --- w amax on DVE as tiles land (rings interleave even/odd
            # landings): singles first so the chain starts early, pairs
            # after (one instruction-overhead per two tiles) ---
            def amax(dst, src):
                nc.vector.reduce_max(
                    dst, src, axis=X, apply_absolute_value=True
                )

            for dd in (0, 1):
                amax(amw_parts[:, dd : dd + 1], w_sb[:, dd, :])
            for p0 in (1, 2, 3):
                amax(amw_parts[:, 2 * p0 : 2 * p0 + 2], w_sb[:, 2 * p0 : 2 * p0 + 2, :])

            # --- w: exact f32 transpose (PE transpose-mode) during the load
            # window (no amax dependency), staged to wT in f32 via ACT ---
            for dd in range(ND):
                pwf = pt.tile([P, NK, P], dt.float32, tag="pt")
                for kk in range(NK):
                    nc.tensor.transpose(
                        pwf[:, kk, :], w_sb[:, dd, kk * P : (kk + 1) * P], id_f32[:]
                    )
                nc.scalar.copy(wT[:, :, dd, :], pwf[:])

            # --- rw chain (recip emitted later, after PAR has its input) ---
            nc.vector.reduce_max(amw_c[:], amw_parts[:], axis=X)
            nc.vector.tensor_scalar_max(amw_c[:], amw_c[:], EPS)
            nc.gpsimd.partition_all_reduce(
                amw_g[:], amw_c[:], channels=P, reduce_op=bass_isa.ReduceOp.max
            )

            # --- x chain.  Per tile: DVE accum-amax + rx (eps/recip/x224);
            # m_all (needs cw) is batched later.  Quant on ACT, fp8 PE
            # transpose, evicts: t0-t4 on ACT, t5-t7 on DVE (post-amax). ---
            def rx_chain(sl):
                nc.vector.tensor_scalar_max(amx_cl[:, sl], amx_parts[:, sl], EPS)
                nc.vector.reciprocal(inv_x[:, sl], amx_cl[:, sl])
                nc.vector.tensor_scalar_mul(rx[:, sl], inv_x[:, sl], E4M3_MAX / 2.0)

            def x_quant(tt):
                nc.scalar.mul(qx[:, tt, :], x_sb[:, tt, :], rx[:, tt : tt + 1])

            def x_transpose(tt):
                pxf = pt.tile([P, NK, P], dt.float32, tag="pt")
                for kk in range(NK):
                    nc.tensor.matmul(
                        pxf[:, kk, :],
                        lhsT=qx[:, tt, kk * P : (kk + 1) * P],
                        rhs=id_fp8[:],
                        start=True, stop=True,
                    )
                return pxf

            def x_evict(tt, pxf, eng):
                if eng == "v":
                    nc.vector.tensor_copy(qxT[:, :, tt, :], pxf[:])
                else:
                    nc.scalar.copy(qxT[:, :, tt, :], pxf[:])

            # t0, t1 singles with rx right away (pipe startup)
            amax(amx_parts[:, 0:1], x_sb[:, 0, :])
            rx_chain(slice(0, 1))
            amax(amx_parts[:, 1:2], x_sb[:, 1, :])
            rx_chain(slice(1, 2))
            # rw tail on DVE (PAR done on gpsimd by now) + w quant kk0,1
            nc.vector.reciprocal(inv_w[:], amw_g[:])
            nc.vector.tensor_scalar_mul(rw[:], inv_w[:], E4M3_MAX / 2.0)
            nc.vector.tensor_scalar_mul(cw[:], amw_g[:], 4.0 / (E4M3_MAX * E4M3_MAX))
            nc.vector.tensor_scalar_mul(qwT[:, 0:2, :, :], wT[:, 0:2, :, :], rw[:])

            # ACT: x quants for t0/t1 + w quant pairs interleaved with evicts
            x_quant(0)
            x_quant(1)
            nc.scalar.mul(qwT[:, 2:4, :, :], wT[:, 2:4, :, :], rw[:])
            pxf0 = x_transpose(0)
            x_evict(0, pxf0, "s")
            nc.scalar.mul(qwT[:, 4:6, :, :], wT[:, 4:6, :, :], rw[:])
            pxf1 = x_transpose(1)
            x_evict(1, pxf1, "s")
            nc.scalar.mul(qwT[:, 6:8, :, :], wT[:, 6:8, :, :], rw[:])

            # remaining x tiles: amax pairs + rx, quant/transpose/evict
            for pr in (1, 2, 3):
                t0 = 2 * pr
                sl = slice(t0, t0 + 2)
                amax(amx_parts[:, sl], x_sb[:, sl, :])
                rx_chain(sl)
                for tt in (t0, t0 + 1):
                    x_quant(tt)
                    pxf = x_transpose(tt)
                    x_evict(tt, pxf, "s" if tt < 6 else "v")

            # m_all batched (single set of ops once cw exists)
            nc.vector.tensor_scalar(
                m_all[:], amx_cl[:], cw[:], None, op0=ALU.mult
            )

            # --- main fp8 DoubleRow matmul: out[t,d] accumulated over the
            # four k-pairs in quant-completion order ---
            for tt in range(NT):
                po = pm.tile([P, DOUT], dt.float32, tag="pm")
                for kp in range(4):
                    ks = slice(2 * kp, 2 * kp + 2)
                    st, sp = kp == 0, kp == 3
                    for h in range(2):
                        nc.tensor.matmul(
                            po[:, h * 512 : (h + 1) * 512],
                            lhsT=qxT[:, ks, tt, :],
                            rhs=qwT[:, ks, 4 * h : 4 * h + 4, :],
                            start=st, stop=sp,
                            perf_mode=DR,
                        )
                ob = outp.tile([P, DOUT], dt.bfloat16, tag="ob")
                if tt % 2 == 0:
                    nc.scalar.mul(ob[:], po[:], m_all[:, tt : tt + 1])
                else:
                    nc.vector.tensor_scalar_mul(ob[:], po[:], m_all[:, tt : tt + 1])
                nc.sync.dma_start(o_d[:, tt, :], ob[:])

    nc.compile()
    return nc


def get_nc():
    if "nc" not in _CACHE:
        _CACHE["nc"] = _build_nc()
    return _CACHE["nc"]


def make_in_maps(x, weight):
    x = np.ascontiguousarray(np.asarray(x, dtype=np.float32))
    w = np.ascontiguousarray(np.asarray(weight, dtype=np.float32))
    return [
        {"x": x[TPE * e : TPE * (e + 1)], "w": w[DOUT * e : DOUT * (e + 1)]}
        for e in range(NE)
    ]


def _host_reference(x, weight, tokens_per_expert):
    """Exact numpy port of the reference — fallback for non-uniform routing."""
    x = np.asarray(x, dtype=np.float32)
    w = np.asarray(weight, dtype=np.float32)
    tpe = np.asarray(tokens_per_expert, dtype=np.int64)
    ne = tpe.shape[0]
    T, din = x.shape
    dout = w.shape[0] // ne
    wr = w.reshape(ne, dout, din)

    def qd(v, axis, fmax):
        amax = np.max(np.abs(v), axis=axis, keepdims=True)
        scale = np.maximum(amax, EPS) / fmax
        q = np.clip(v / scale, -fmax, fmax).astype(ml_dtypes.float8_e4m3fn)
        return q.astype(np.float32) * scale

    w_dq = qd(wr, (1, 2), E4M3_MAX)
    x_dq = qd(x, -1, E4M3_MAX)
    offs = np.cumsum(tpe)
    starts = offs - tpe
    out = np.zeros((T, dout), np.float32)
    for e in range(ne):
        s, t = int(starts[e]), int(offs[e])
        if t > s:
            out[s:t] = x_dq[s:t] @ w_dq[e].T
    return out.astype(ml_dtypes.bfloat16)


def kernel(x, weight, tokens_per_expert):
    x = np.asarray(x)
    weight = np.asarray(weight)
    tpe = np.asarray(tokens_per_expert)
    uniform = (
        x.shape == (NE * TPE, DIN)
        and weight.shape == (NE * DOUT, DIN)
        and tpe.shape == (NE,)
        and bool(np.all(tpe.astype(np.int64) == TPE))
    )
    if not uniform:
        return _host_reference(x, weight, tpe)

    from concourse.bass_utils import run_bass_kernel_spmd

    nc = get_nc()
    in_maps = make_in_maps(x, weight)
    try:
        res = run_bass_kernel_spmd(nc, in_maps, core_ids=list(range(NE)))
    except Exception:
        # rare device wedge (NRT_EXEC_UNIT_UNRECOVERABLE) — reset and retry
        _axon_device_reset()
        res = run_bass_kernel_spmd(nc, in_maps, core_ids=list(range(NE)))
    return np.concatenate([res.results[e]["o"] for e in range(NE)], axis=0)


if __name__ == "__main__":
    rng = np.random.default_rng(0)
    x = rng.standard_normal((NE * TPE, DIN), dtype=np.float32)
    w = (rng.standard_normal((NE * DOUT, DIN), dtype=np.float32) * 0.02).astype(
        np.float32
    )
    tpe = np.full((NE,), TPE, dtype=np.int32)
    out = kernel(x, w, tpe)
    exp = _host_reference(x, w, tpe)
    a = out.astype(np.float64)
    b = exp.astype(np.float64)
    denom = max(np.abs(b).max(), 1e-30)
    print("absmax rel err:", np.abs(a - b).max() / denom)
    rms = np.sqrt(((a - b) ** 2).mean()) / np.sqrt((b**2).mean())
    print("rms rel err:", rms)


# revision 14
# speedup vs baseline: 3.0447x; 1.0843x over previous
"""ChannelWiseFloat8GroupedLinear — expert-parallel Trainium2 Bass kernel.

Problem: x [8192, 1024] f32, weight [8*1024, 1024] f32, tokens_per_expert
[8] int32 (uniform 1024).  out[t, d] = x_dq @ w_dq[e(t)].T in bf16, where
x is fp8-e4m3fn quant-dequantized per token row and w per expert block.

Sharding: expert-parallel over 8 NeuronCores.  Tokens are contiguous per
expert (cumsum offsets), so core e owns x rows [1024e, 1024e+1024) and
expert e's weight block — no cross-core communication.

Device math: the reference quantizes to OCP e4m3fn (max 448); TRN2's
fp8_e4m3 tops out at 240.  Quantizing with r = 224/amax instead of
448/amax lands on the halved e4m3fn grid, which TRN e4m3 represents
exactly (up to a negligible subnormal-spacing difference), and the x4
is folded into the output scale m[t] = amax_x[t]*amax_w/(448*448/4).
The fp8 matmul accumulates exact products in f32 PSUM, so the result
matches the reference to ~f32 rounding before the final bf16 cast.

Schedule (v2): all input DMAs ride the sync HWDGE ring w-first (ring is
FIFO, so w gets full bandwidth, then x streams behind it).  While w
loads, PE transposes it in f32 and ACT evicts to SBUF; DVE reduces the
global |w| amax.  Once rw is known, w quant is split DVE/ACT/GPSIMD per
k-slice and the main matmul consumes k-pairs in quant-completion order.
x tiles pipeline per-tile: DVE amax -> GPSIMD quant -> PE fp8 transpose
-> DVE/ACT evict -> DoubleRow fp8 matmul -> scaled bf16 evict -> store.
"""

import numpy as np
import ml_dtypes

P = 128
TPE = 1024   # tokens per expert (= T // ne, uniform)
DIN = 1024
DOUT = 1024
NE = 8
NT = TPE // P    # 8 token tiles per core
ND = DOUT // P   # 8 dout tiles per core
NK = DIN // P    # 8 contraction tiles
E4M3_MAX = 448.0
EPS = 1e-12

_CACHE = {}


def _axon_device_reset():
    """Best-effort reset of the axon-tunneled NeuronCores after an
    NRT_EXEC_UNIT_UNRECOVERABLE wedge (observed rarely; a reset recovers)."""
    try:
        import ctypes

        import jax

        jax.devices()
        lib = ctypes.CDLL("/opt/axon/libaxon_pjrt.so")
        if hasattr(lib, "axon_reset"):
            lib.axon_reset.restype = ctypes.c_int64
            lib.axon_reset()
    except Exception:
        pass


def _build_nc():
    """Build + compile the single-core Bass program (run SPMD on 8 cores)."""
    import concourse.mybir as mybir
    import concourse.tile as tile
    from concourse import bacc, bass_isa
    from concourse.masks import make_identity

    dt = mybir.dt
    X = mybir.AxisListType.X
    ALU = mybir.AluOpType
    DR = mybir.MatmulPerfMode.DoubleRow

    nc = bacc.Bacc("TRN2", target_bir_lowering=False, debug=False)
    x_t = nc.dram_tensor("x", [TPE, DIN], dt.float32, kind="ExternalInput")
    w_t = nc.dram_tensor("w", [DOUT, DIN], dt.float32, kind="ExternalInput")
    o_t = nc.dram_tensor("o", [TPE, DOUT], dt.bfloat16, kind="ExternalOutput")

    x_d = x_t.ap().rearrange("(tt p) k -> p tt k", p=P)   # [128, 8, 1024]
    w_d = w_t.ap().rearrange("(dd p) k -> p dd k", p=P)
    o_d = o_t.ap().rearrange("(tt p) d -> p tt d", p=P)

    with tile.TileContext(nc) as tc:
        with (
            tc.tile_pool(name="const", bufs=1) as const,
            tc.tile_pool(name="big", bufs=1) as big,
            tc.tile_pool(name="small", bufs=1) as small,
            tc.tile_pool(name="outp", bufs=3) as outp,
            tc.tile_pool(name="pt", bufs=2, space="PSUM") as pt,
            tc.tile_pool(name="pm", bufs=2, space="PSUM") as pm,
        ):
            # persistent buffers
            x_sb = big.tile([P, NT, DIN], dt.float32, tag="x_sb")
            w_sb = big.tile([P, ND, DIN], dt.float32, tag="w_sb")
            wT = big.tile([P, NK, ND, P], dt.float32, tag="wT")
            qx = big.tile([P, NT, DIN], dt.float8e4, tag="qx")
            qxT = big.tile([P, NK, NT, P], dt.float8e4, tag="qxT")
            qwT = big.tile([P, NK, ND, P], dt.float8e4, tag="qwT")

            amw_parts = small.tile([P, ND], dt.float32, tag="amw_parts")
            amw_c = small.tile([P, 1], dt.float32, tag="amw_c")
            amw_g = small.tile([P, 1], dt.float32, tag="amw_g")
            inv_w = small.tile([P, 1], dt.float32, tag="inv_w")
            rw = small.tile([P, 1], dt.float32, tag="rw")
            cw = small.tile([P, 1], dt.float32, tag="cw")
            amx_parts = small.tile([P, NT], dt.float32, tag="amx_parts")
            amx_cl = small.tile([P, NT], dt.float32, tag="amx_cl")
            inv_x = small.tile([P, NT], dt.float32, tag="inv_x")
            rx = small.tile([P, NT], dt.float32, tag="rx")
            m_all = small.tile([P, NT], dt.float32, tag="m_all")
            junk = small.tile([P, 1], dt.float32, tag="junk")

            # --- input DMAs.  A single HWDGE ring only sustains ~250 GB/s
            # (FIFO gap between consecutive DMAs), so split each stream
            # across both HWDGE rings (sync + scalar).  Each ring is FIFO,
            # so w transfers strictly precede x transfers per ring; x-odds
            # ride SWDGE (gpsimd), gated on w completion by a cheap native
            # partition_all_reduce that reads the last w tile. ---
            for i in (0, 2, 4, 6):
                nc.sync.dma_start(w_sb[:, i, :], w_d[:, i, :])
            for i in (1, 3, 5, 7):
                nc.scalar.dma_start(w_sb[:, i, :], w_d[:, i, :])
            for i in (0, 2, 4, 6):
                nc.sync.dma_start(x_sb[:, i, :], x_d[:, i, :])
            # x-odd triggers are emitted inside the w-evict loop below so
            # their sem-waits don't stall ACT before the evicts start

            # transpose identities (fp8: 1.0 is exactly representable)
            id_f32 = const.tile([P, P], dt.float32, tag="id32f")
            make_identity(nc, id_f32[:])
            id_fp8 = const.tile([P, P], dt.float8e4, tag="id8")
            nc.vector.tensor_copy(id_fp8[:], id_f32[:])

            _ = junk

            # --- w amax on DVE as tiles land (rings interleave even/odd
            # landings): singles first so the chain starts early, pairs
            # after (one instruction-overhead per two tiles) ---
            def amax(dst, src):
                nc.vector.reduce_max(
                    dst, src, axis=X, apply_absolute_value=True
                )

            for dd in (0, 1):
                amax(amw_parts[:, dd : dd + 1], w_sb[:, dd, :])
            for p0 in (1, 2, 3):
                amax(amw_parts[:, 2 * p0 : 2 * p0 + 2], w_sb[:, 2 * p0 : 2 * p0 + 2, :])

            # --- w: exact f32 transpose (PE transpose-mode) during the load
            # window (no amax dependency), staged to wT in f32 via ACT ---
            for dd in range(ND):
                pwf = pt.tile([P, NK, P], dt.float32, tag="pt")
                for kk in range(NK):
                    nc.tensor.transpose(
                        pwf[:, kk, :], w_sb[:, dd, kk * P : (kk + 1) * P], id_f32[:]
                    )
                nc.scalar.copy(wT[:, :, dd, :], pwf[:])
                if dd < 4:
                    nc.scalar.dma_start(
                        x_sb[:, 2 * dd + 1, :], x_d[:, 2 * dd + 1, :]
                    )

            # --- rw chain (recip emitted later, after PAR has its input) ---
            nc.vector.reduce_max(amw_c[:], amw_parts[:], axis=X)
            nc.vector.tensor_scalar_max(amw_c[:], amw_c[:], EPS)
            nc.gpsimd.partition_all_reduce(
                amw_g[:], amw_c[:], channels=P, reduce_op=bass_isa.ReduceOp.max
            )

            # --- x chain.  Per tile: DVE accum-amax + rx (eps/recip/x224);
            # m_all (needs cw) is batched later.  Quant on ACT, fp8 PE
            # transpose, evicts: t0-t4 on ACT, t5-t7 on DVE (post-amax). ---
            def rx_chain(sl):
                nc.vector.tensor_scalar_max(amx_cl[:, sl], amx_parts[:, sl], EPS)
                nc.vector.reciprocal(inv_x[:, sl], amx_cl[:, sl])
                nc.vector.tensor_scalar_mul(rx[:, sl], inv_x[:, sl], E4M3_MAX / 2.0)

            def x_quant(tt):
                nc.scalar.mul(qx[:, tt, :], x_sb[:, tt, :], rx[:, tt : tt + 1])

            def x_transpose(tt):
                pxf = pt.tile([P, NK, P], dt.float32, tag="pt")
                for kk in range(NK):
                    nc.tensor.matmul(
                        pxf[:, kk, :],
                        lhsT=qx[:, tt, kk * P : (kk + 1) * P],
                        rhs=id_fp8[:],
                        start=True, stop=True,
                    )
                return pxf

            def x_evict(tt, pxf, eng):
                if eng == "v":
                    nc.vector.tensor_copy(qxT[:, :, tt, :], pxf[:])
                else:
                    nc.scalar.copy(qxT[:, :, tt, :], pxf[:])

            # t0, t1 singles with rx right away (pipe startup)
            amax(amx_parts[:, 0:1], x_sb[:, 0, :])
            rx_chain(slice(0, 1))
            amax(amx_parts[:, 1:2], x_sb[:, 1, :])
            rx_chain(slice(1, 2))
            # rw tail on DVE (PAR done on gpsimd by now) + w quant kk0,1
            nc.vector.reciprocal(inv_w[:], amw_g[:])
            nc.vector.tensor_scalar_mul(rw[:], inv_w[:], E4M3_MAX / 2.0)
            nc.vector.tensor_scalar_mul(cw[:], amw_g[:], 4.0 / (E4M3_MAX * E4M3_MAX))
            nc.vector.tensor_scalar_mul(qwT[:, 0:2, :, :], wT[:, 0:2, :, :], rw[:])

            # ACT: x quants for t0/t1 + w quant pairs interleaved with evicts
            x_quant(0)
            x_quant(1)
            nc.scalar.mul(qwT[:, 2:4, :, :], wT[:, 2:4, :, :], rw[:])
            pxf0 = x_transpose(0)
            x_evict(0, pxf0, "s")
            nc.scalar.mul(qwT[:, 4:6, :, :], wT[:, 4:6, :, :], rw[:])
            pxf1 = x_transpose(1)
            x_evict(1, pxf1, "s")
            nc.scalar.mul(qwT[:, 6:8, :, :], wT[:, 6:8, :, :], rw[:])

            # remaining x tiles: amax pairs + rx, quant/transpose/evict
            for pr in (1, 2, 3):
                t0 = 2 * pr
                sl = slice(t0, t0 + 2)
                amax(amx_parts[:, sl], x_sb[:, sl, :])
                rx_chain(sl)
                for tt in (t0, t0 + 1):
                    x_quant(tt)
                    pxf = x_transpose(tt)
                    x_evict(tt, pxf, "s" if tt < 6 else "v")

            # m_all batched (single set of ops once cw exists)
            nc.vector.tensor_scalar(
                m_all[:], amx_cl[:], cw[:], None, op0=ALU.mult
            )

            # --- main fp8 DoubleRow matmul: out[t,d] accumulated over the
            # four k-pairs in quant-completion order ---
            for tt in range(NT):
                po = pm.tile([P, DOUT], dt.float32, tag="pm")
                for kp in range(4):
                    ks = slice(2 * kp, 2 * kp + 2)
                    st, sp = kp == 0, kp == 3
                    for h in range(2):
                        nc.tensor.matmul(
                            po[:, h * 512 : (h + 1) * 512],
                            lhsT=qxT[:, ks, tt, :],
                            rhs=qwT[:, ks, 4 * h : 4 * h + 4, :],
                            start=st, stop=sp,
                            perf_mode=DR,
                        )
                ob = outp.tile([P, DOUT], dt.bfloat16, tag="ob")
                if tt % 2 == 0:
                    nc.scalar.mul(ob[:], po[:], m_all[:, tt : tt + 1])
                else:
                    nc.vector.tensor_scalar_mul(ob[:], po[:], m_all[:, tt : tt + 1])
                nc.sync.dma_start(o_d[:, tt, :], ob[:])

    nc.compile()
    return nc


def get_nc():
    if "nc" not in _CACHE:
        _CACHE["nc"] = _build_nc()
    return _CACHE["nc"]


def make_in_maps(x, weight):
    x = np.ascontiguousarray(np.asarray(x, dtype=np.float32))
    w = np.ascontiguousarray(np.asarray(weight, dtype=np.float32))
    return [
        {"x": x[TPE * e : TPE * (e + 1)], "w": w[DOUT * e : DOUT * (e + 1)]}
        for e in range(NE)
    ]


def _host_reference(x, weight, tokens_per_expert):
    """Exact numpy port of the reference — fallback for non-uniform routing."""
    x = np.asarray(x, dtype=np.float32)
    w = np.asarray(weight, dtype=np.float32)
    tpe = np.asarray(tokens_per_expert, dtype=np.int64)
    ne = tpe.shape[0]
    T, din = x.shape
    dout = w.shape[0] // ne
    wr = w.reshape(ne, dout, din)

    def qd(v, axis, fmax):
        amax = np.max(np.abs(v), axis=axis, keepdims=True)
        scale = np.maximum(amax, EPS) / fmax
        q = np.clip(v / scale, -fmax, fmax).astype(ml_dtypes.float8_e4m3fn)
        return q.astype(np.float32) * scale

    w_dq = qd(wr, (1, 2), E4M3_MAX)
    x_dq = qd(x, -1, E4M3_MAX)
    offs = np.cumsum(tpe)
    starts = offs - tpe
    out = np.zeros((T, dout), np.float32)
    for e in range(ne):
        s, t = int(starts[e]), int(offs[e])
        if t > s:
            out[s:t] = x_dq[s:t] @ w_dq[e].T
    return out.astype(ml_dtypes.bfloat16)


def kernel(x, weight, tokens_per_expert):
    x = np.asarray(x)
    weight = np.asarray(weight)
    tpe = np.asarray(tokens_per_expert)
    uniform = (
        x.shape == (NE * TPE, DIN)
        and weight.shape == (NE * DOUT, DIN)
        and tpe.shape == (NE,)
        and bool(np.all(tpe.astype(np.int64) == TPE))
    )
    if not uniform:
        return _host_reference(x, weight, tpe)

    from concourse.bass_utils import run_bass_kernel_spmd

    nc = get_nc()
    in_maps = make_in_maps(x, weight)
    try:
        res = run_bass_kernel_spmd(nc, in_maps, core_ids=list(range(NE)))
    except Exception:
        # rare device wedge (NRT_EXEC_UNIT_UNRECOVERABLE) — reset and retry
        _axon_device_reset()
        res = run_bass_kernel_spmd(nc, in_maps, core_ids=list(range(NE)))
    return np.concatenate([res.results[e]["o"] for e in range(NE)], axis=0)


if __name__ == "__main__":
    rng = np.random.default_rng(0)
    x = rng.standard_normal((NE * TPE, DIN), dtype=np.float32)
    w = (rng.standard_normal((NE * DOUT, DIN), dtype=np.float32) * 0.02).astype(
        np.float32
    )
    tpe = np.full((NE,), TPE, dtype=np.int32)
    out = kernel(x, w, tpe)
    exp = _host_reference(x, w, tpe)
    a = out.astype(np.float64)
    b = exp.astype(np.float64)
    denom = max(np.abs(b).max(), 1e-30)
    print("absmax rel err:", np.abs(a - b).max() / denom)
    rms = np.sqrt(((a - b) ** 2).mean()) / np.sqrt((b**2).mean())
    print("rms rel err:", rms)
